# revision 1
# baseline (speedup 1.0000x reference)
"""Trainium2 Bass kernel for the FallbackSSMKernel problem.

Computation (reference):
    xz = hidden @ W_in.T                     # (B,S,2*INNER)
    x, z = split(xz);  x -> (B,S,H,DH)
    h_n = A*h_{n-1} + 0.1*x_n  over chunks of 256 positions (per head)
    y = scan_out * sigmoid(z)
    out = y @ W_out.T                        # (B,S,DM)

Sharding: 8-way tensor-parallel by heads (14 heads / 896 inner dims per
core).  Each core computes the full token range for its head slice and a
partial out-projection (contraction over its 896 inner dims); partials
are summed on the host.

Device layout is fully transposed: activations live as (feature, token)
with features on SBUF partitions, so no transposes are ever needed on
device.  Per 512-token group the kernel runs:
  A) in_proj:  PSUM tiles (x_s/z_s pairs), each accumulated over the 28
     K-tiles of 128 (bf16 matmuls, fp32 PSUM accumulation)
  B) scan+gate directly from PSUM: h = h*A + x (DVE scalar_tensor_tensor,
     fp32 state), sig = sigmoid(z) (ACT), y = h*sig -> bf16 SBUF
  C) out_proj (transposed): outT[dm_tile, tokens] accumulated over the 7
     inner K-tiles, evicted to DRAM fp32.

The 0.1 scan input scale is folded into the x-rows of W_in on the host.
"""

import numpy as np
import ml_dtypes

B, S, DM = 2, 4096, 3584
H, DH = 112, 64
CHUNK = 256
INNER = H * DH
N_CORES = 8
HPC = H // N_CORES          # heads per core = 14
ISL = HPC * DH              # inner slice per core = 896
T = B * S                   # total tokens = 8192
G = 512                     # tokens per group

BF16 = ml_dtypes.bfloat16

_nc_cache = {}


def _patch_tile_drain():
    """Split the Tile end-of-kernel drain's semaphore waits across NOPs.

    The walrus build here rejects an InstDrain carrying more than a
    couple of sync waits ("Too many sync wait commands" in
    CoreV3GenImpl::setupSyncWait).  TileContext._drain_and_barrier
    attaches one wait per outstanding logical processor to the single
    drain, which trips that limit for any kernel that used a few DMA
    queues.  Emit one single-wait NOP per processor first so the drain
    itself needs no waits.
    """
    import concourse.tile as tile
    from concourse.vector_clock import ScopedClock, VectorClock

    if getattr(tile.TileContext, "_drain_split_patched", False):
        return

    def _drain_and_barrier(self, tick_clock, wait_clock):
        full = tick_clock.global_clock
        n = len(full)
        for proc in range(n):
            t = full[proc]
            if t > 0:
                vec = [0] * n
                vec[proc] = t
                nop = self.nc.sync.nop(nofuse=True, hint="drain_split")
                wait_clock.add_sem_waits(nop.ins, ScopedClock({None: VectorClock(vec)}))
        # No waits on the drain itself: SP executes the single-wait NOPs
        # above in order first, so every processor's final tick has been
        # observed before the drain runs.
        self.nc.sync.drain()
        self.nc.all_engine_barrier()
        popped = self.nc._tile_sem_poison_stack.pop()
        assert popped is self._sem_poison
        self.nc.clear_and_free_semaphores(list(self.sems.allocated().values()))
        self.nc.all_engine_barrier()

    tile.TileContext._drain_and_barrier = _drain_and_barrier
    tile.TileContext._drain_split_patched = True


def _split_excess_waits(nc, limit=1):
    """Hoist excess per-instruction semaphore waits onto inserted NOPs.

    The TRN2 64-byte instruction encoding carries at most `limit` sync
    waits; this walrus build hard-errors on more.  Tile can attach 3+
    waits to one instruction.  Hoisting the earliest waits onto
    preceding same-engine NOPs is semantics-preserving: semaphore
    values are monotonic, so waiting earlier on the same engine keeps
    the ordering guarantees.
    """
    import concourse.mybir as mybir

    counter = [0]
    for f in nc.m.functions:
        for blk in f.blocks:
            insts = blk.instructions
            new = []
            changed = False
            for inst in insts:
                si = inst.sync_info
                if si is not None and si.on_wait and len(si.on_wait) > limit:
                    waits = list(si.on_wait)
                    extra, keep = waits[:-limit], waits[-limit:]
                    for i in range(0, len(extra), limit):
                        chunk_w = extra[i:i + limit]
                        nop = mybir.InstNoOp(
                            name=f"WSPLIT-{counter[0]}", ins=[], outs=[]
                        )
                        counter[0] += 1
                        nop.engine = inst.engine
                        nop.sync_info = mybir.SyncInfo(
                            on_wait=chunk_w, on_update=[]
                        )
                        new.append(nop)
                    si.on_wait = keep
                    changed = True
                new.append(inst)
            if changed:
                blk.instructions = new
    return counter[0]


def _build_bass(dm=DM, isl=ISL, tokens=T, n_batch=B, group=G, chunk=CHUNK,
                repeat=1, psa_bufs=4, psc_bufs=3, z_first=False):
    """Build the per-core Bass module.

    Inputs (per core):
      hid_t  (dm, tokens)  bf16 : hidden_states, transposed
      w_in_t (dm, 2*isl)   bf16 : in_proj weight shard, transposed;
                                  cols [0,isl) are x-rows (pre-scaled by
                                  0.1), cols [isl,2*isl) are z-rows
      w_out_t(isl, dm)     bf16 : out_proj weight shard, transposed
      a_vec  (128, isl/128) f32 : per-inner-dim decay A = exp(-|A_log|)
    Output:
      out_t  (dm, tokens)  f32 : partial out-projection, transposed
    """
    import concourse.bass as bass
    import concourse.tile as tile
    import concourse.mybir as mybir

    _patch_tile_drain()

    ka = dm // 128            # in_proj contraction tiles
    kc = isl // 128           # out_proj contraction tiles / x slabs
    ng = tokens // group      # token groups
    gpb = ng // n_batch       # groups per batch
    kh = ka // 2              # k-tiles per hidden half-slab
    cpg = group // chunk      # chunks per group

    nc = bass.Bass("TRN2")
    dt = mybir.dt

    hid = nc.dram_tensor("hid_t", (dm, tokens), dt.bfloat16, kind="ExternalInput")
    w_in = nc.dram_tensor("w_in_t", (dm, 2 * isl), dt.bfloat16, kind="ExternalInput")
    w_out = nc.dram_tensor("w_out_t", (isl, dm), dt.bfloat16, kind="ExternalInput")
    a_vec = nc.dram_tensor("a_vec", (128, kc), dt.float32, kind="ExternalInput")
    out = nc.dram_tensor("out_t", (dm, tokens), dt.float32, kind="ExternalOutput")

    hid_r = hid[:].rearrange("(k p) t -> p k t", p=128)
    win_r = w_in[:].rearrange("(k p) m -> p k m", p=128)
    wout_r = w_out[:].rearrange("(k p) n -> p k n", p=128)

    with tile.TileContext(nc) as tc:
        with (
            tc.tile_pool(name="w_in", bufs=1) as p_win,
            tc.tile_pool(name="w_out", bufs=1) as p_wout,
            tc.tile_pool(name="consts", bufs=1) as p_const,
            tc.tile_pool(name="hid", bufs=2) as p_hid,
            tc.tile_pool(name="ysb", bufs=2) as p_y,
            tc.tile_pool(name="hstate", bufs=1) as p_h,
            tc.tile_pool(name="sig", bufs=2) as p_sig,
            tc.tile_pool(name="oev", bufs=2) as p_oev,
            tc.tile_pool(name="psA", bufs=psa_bufs, space="PSUM") as p_psA,
            tc.tile_pool(name="psC", bufs=psc_bufs, space="PSUM") as p_psC,
        ):
            win_sb = p_win.tile([128, ka, 2 * isl], dt.bfloat16)
            for k in range(ka):
                nc.sync.dma_start(out=win_sb[:, k, :], in_=win_r[:, k, :])
            wout_sb = p_wout.tile([128, kc, dm], dt.bfloat16)
            for k in range(kc):
                nc.sync.dma_start(out=wout_sb[:, k, :], in_=wout_r[:, k, :])
            a_sb = p_const.tile([128, kc], dt.float32)
            nc.sync.dma_start(out=a_sb[:], in_=a_vec[:])
            h_sb = p_h.tile([128, kc, chunk], dt.float32)

            import contextlib
            rep_ctx = (
                tc.For_i(0, repeat, 1) if repeat > 1 else contextlib.nullcontext()
            )
            with rep_ctx:
                _emit_groups(
                    nc, tc, mybir, ng, gpb, ka, kh, kc, cpg, dm, isl, group, chunk,
                    hid_r, out, p_hid, p_y, p_sig, p_oev, p_psA, p_psC,
                    win_sb, wout_sb, a_sb, h_sb, z_first=z_first,
                )

    n_split = _split_excess_waits(nc)
    if n_split:
        print(f"_split_excess_waits: inserted {n_split} NOPs", flush=True)
    return nc


def _emit_groups(nc, tc, mybir, ng, gpb, ka, kh, kc, cpg, dm, isl, group, chunk,
                 hid_r, out, p_hid, p_y, p_sig, p_oev, p_psA, p_psC,
                 win_sb, wout_sb, a_sb, h_sb, pipeline=False, z_first=False):
    # pipeline=True (out_proj of group g-1 emitted after in_proj of group
    # g) measured model-neutral and hung on hardware — keep it off.
    dt = mybir.dt

    def emit_phase_c(g, y_sb):
        for m in range(dm // 128):
            po = p_psC.tile([128, group], dt.float32, tag="po")
            for k in range(kc):
                nc.tensor.matmul(
                    po,
                    wout_sb[:, k, m * 128:(m + 1) * 128],
                    y_sb[:, k, :],
                    start=(k == 0),
                    stop=(k == kc - 1),
                )
            oev = p_oev.tile([128, group], dt.float32, tag="oev")
            nc.vector.tensor_copy(oev[:], po[:])
            nc.sync.dma_start(
                out=out[m * 128:(m + 1) * 128, g * group:(g + 1) * group],
                in_=oev[:],
            )

    pending_c = None
    if True:
        if True:
            for g in range(ng):
                if g % gpb == 0:
                    # scan state resets at each batch boundary
                    nc.vector.memset(h_sb[:], 0.0)

                hid_t = []
                for hh in range(2):
                    ht = p_hid.tile([128, kh, group], dt.bfloat16, tag="hid")
                    nc.sync.dma_start(
                        out=ht[:],
                        in_=hid_r[:, hh * kh:(hh + 1) * kh, g * group:(g + 1) * group],
                    )
                    hid_t.append(ht)

                y_sb = p_y.tile([128, kc, group], dt.bfloat16)

                for s in range(kc):
                    px = p_psA.tile([128, group], dt.float32, tag="ps")
                    pz = p_psA.tile([128, group], dt.float32, tag="ps")

                    def emit_x():
                        for k in range(ka):
                            hh, kk = divmod(k, kh)
                            nc.tensor.matmul(
                                px,
                                win_sb[:, k, s * 128:(s + 1) * 128],
                                hid_t[hh][:, kk, :],
                                start=(k == 0),
                                stop=(k == ka - 1),
                            )

                    def emit_z():
                        for k in range(ka):
                            hh, kk = divmod(k, kh)
                            nc.tensor.matmul(
                                pz,
                                win_sb[:, k, isl + s * 128:isl + (s + 1) * 128],
                                hid_t[hh][:, kk, :],
                                start=(k == 0),
                                stop=(k == ka - 1),
                            )

                    if z_first:
                        emit_z()
                        emit_x()
                    else:
                        emit_x()
                        emit_z()
                    sig = p_sig.tile([128, group], dt.bfloat16, tag="sig")
                    nc.scalar.activation(
                        sig[:], pz[:], mybir.ActivationFunctionType.Sigmoid
                    )
                    for c in range(cpg):
                        cs = slice(c * chunk, (c + 1) * chunk)
                        nc.vector.scalar_tensor_tensor(
                            out=h_sb[:, s, :],
                            in0=h_sb[:, s, :],
                            scalar=a_sb[:, s:s + 1],
                            in1=px[:, cs],
                            op0=mybir.AluOpType.mult,
                            op1=mybir.AluOpType.add,
                        )
                        nc.vector.tensor_mul(y_sb[:, s, cs], h_sb[:, s, :], sig[:, cs])

                if pipeline:
                    # depth-1 software pipeline: group g-1's out_proj runs
                    # after group g's in_proj on the PE, so the PE never
                    # waits on the DVE scan/gate chain.
                    if pending_c is not None:
                        emit_phase_c(*pending_c)
                    pending_c = (g, y_sb)
                else:
                    emit_phase_c(g, y_sb)
            if pipeline and pending_c is not None:
                emit_phase_c(*pending_c)
                pending_c = None


# Results of the most recent device run (for test harness introspection).
last_result = None


def _prep_core_inputs(hidden_states, W_in, W_out, A_log):
    """Host-side shard prep. Returns (in_maps, hid_t shared array)."""
    hid_t = np.ascontiguousarray(hidden_states.reshape(T, DM).T).astype(BF16)
    A_full = np.exp(-np.abs(A_log)).astype(np.float32)  # (H,)

    in_maps = []
    for c in range(N_CORES):
        isl_sl = slice(c * ISL, (c + 1) * ISL)
        # fold the 0.1 scan input scale into the x-rows of W_in
        w_x = W_in[:INNER][isl_sl] * np.float32(0.1)
        w_z = W_in[INNER:][isl_sl]
        w_in_t = np.ascontiguousarray(
            np.concatenate([w_x, w_z], axis=0).T
        ).astype(BF16)  # (DM, 2*ISL)
        w_out_t = np.ascontiguousarray(W_out[:, isl_sl].T).astype(BF16)  # (ISL, DM)
        a_col = np.repeat(A_full[c * HPC:(c + 1) * HPC], DH)  # (ISL,)
        a_vec = np.ascontiguousarray(
            a_col.reshape(ISL // 128, 128).T
        ).astype(np.float32)  # (128, ISL/128)
        in_maps.append(
            {
                "hid_t": hid_t,
                "w_in_t": w_in_t,
                "w_out_t": w_out_t,
                "a_vec": a_vec,
            }
        )
    return in_maps


def kernel(hidden_states, W_in, W_out, A_log):
    from concourse.bass_utils import run_bass_kernel_spmd

    global last_result

    if "nc" not in _nc_cache:
        _nc_cache["nc"] = _build_bass()
    nc = _nc_cache["nc"]

    hidden_states = np.asarray(hidden_states, dtype=np.float32)
    W_in = np.asarray(W_in, dtype=np.float32)
    W_out = np.asarray(W_out, dtype=np.float32)
    A_log = np.asarray(A_log, dtype=np.float32)

    in_maps = _prep_core_inputs(hidden_states, W_in, W_out, A_log)

    last_result = run_bass_kernel_spmd(nc, in_maps, core_ids=list(range(N_CORES)))

    acc = np.zeros((DM, T), dtype=np.float32)
    for r in last_result.results:
        acc += r["out_t"]
    return np.ascontiguousarray(acc.T).reshape(B, S, DM)


if __name__ == "__main__":
    rng = np.random.default_rng(0)
    ins = {
        "hidden_states": rng.standard_normal((B, S, DM), dtype=np.float32),
        "W_in": (rng.standard_normal((2 * INNER, DM), dtype=np.float32) * 0.02),
        "W_out": (rng.standard_normal((DM, INNER), dtype=np.float32) * 0.02),
        "A_log": rng.standard_normal((H,), dtype=np.float32),
    }
    out = kernel(**ins)
    print(out.shape, out.dtype)



# revision 5
# speedup vs baseline: 1.0109x; 1.0109x over previous
"""Trainium2 Bass kernel for the FallbackSSMKernel problem.

Computation (reference):
    xz = hidden @ W_in.T                     # (B,S,2*INNER)
    x, z = split(xz);  x -> (B,S,H,DH)
    h_n = A*h_{n-1} + 0.1*x_n  over chunks of 256 positions (per head)
    y = scan_out * sigmoid(z)
    out = y @ W_out.T                        # (B,S,DM)

Sharding: 8-way tensor-parallel by heads (14 heads / 896 inner dims per
core).  Each core computes the full token range for its head slice and a
partial out-projection (contraction over its 896 inner dims); partials
are summed on the host.

Device layout is fully transposed: activations live as (feature, token)
with features on SBUF partitions, so no transposes are ever needed on
device.  Per 512-token group the kernel runs:
  A) in_proj:  PSUM tiles (x_s/z_s pairs), each accumulated over the 28
     K-tiles of 128 (bf16 matmuls, fp32 PSUM accumulation)
  B) scan+gate directly from PSUM: h = h*A + x (DVE scalar_tensor_tensor,
     fp32 state), sig = sigmoid(z) (ACT), y = h*sig -> bf16 SBUF
  C) out_proj (transposed): outT[dm_tile, tokens] accumulated over the 7
     inner K-tiles, evicted to DRAM fp32.

The 0.1 scan input scale is folded into the x-rows of W_in on the host.
"""

import numpy as np
import ml_dtypes

B, S, DM = 2, 4096, 3584
H, DH = 112, 64
CHUNK = 256
INNER = H * DH
N_CORES = 8
HPC = H // N_CORES          # heads per core = 14
ISL = HPC * DH              # inner slice per core = 896
T = B * S                   # total tokens = 8192
G = 512                     # tokens per group

BF16 = ml_dtypes.bfloat16

_nc_cache = {}


def _patch_tile_drain():
    """Split the Tile end-of-kernel drain's semaphore waits across NOPs.

    The walrus build here rejects an InstDrain carrying more than a
    couple of sync waits ("Too many sync wait commands" in
    CoreV3GenImpl::setupSyncWait).  TileContext._drain_and_barrier
    attaches one wait per outstanding logical processor to the single
    drain, which trips that limit for any kernel that used a few DMA
    queues.  Emit one single-wait NOP per processor first so the drain
    itself needs no waits.
    """
    import concourse.tile as tile
    from concourse.vector_clock import ScopedClock, VectorClock

    if getattr(tile.TileContext, "_drain_split_patched", False):
        return

    def _drain_and_barrier(self, tick_clock, wait_clock):
        full = tick_clock.global_clock
        n = len(full)
        for proc in range(n):
            t = full[proc]
            if t > 0:
                vec = [0] * n
                vec[proc] = t
                nop = self.nc.sync.nop(nofuse=True, hint="drain_split")
                wait_clock.add_sem_waits(nop.ins, ScopedClock({None: VectorClock(vec)}))
        # No waits on the drain itself: SP executes the single-wait NOPs
        # above in order first, so every processor's final tick has been
        # observed before the drain runs.
        self.nc.sync.drain()
        self.nc.all_engine_barrier()
        popped = self.nc._tile_sem_poison_stack.pop()
        assert popped is self._sem_poison
        self.nc.clear_and_free_semaphores(list(self.sems.allocated().values()))
        self.nc.all_engine_barrier()

    tile.TileContext._drain_and_barrier = _drain_and_barrier
    tile.TileContext._drain_split_patched = True


def _split_excess_waits(nc, limit=1):
    """Hoist excess per-instruction semaphore waits onto inserted NOPs.

    The TRN2 64-byte instruction encoding carries at most `limit` sync
    waits; this walrus build hard-errors on more.  Tile can attach 3+
    waits to one instruction.  Hoisting the earliest waits onto
    preceding same-engine NOPs is semantics-preserving: semaphore
    values are monotonic, so waiting earlier on the same engine keeps
    the ordering guarantees.
    """
    import concourse.mybir as mybir

    counter = [0]
    for f in nc.m.functions:
        for blk in f.blocks:
            insts = blk.instructions
            new = []
            changed = False
            for inst in insts:
                si = inst.sync_info
                if si is not None and si.on_wait and len(si.on_wait) > limit:
                    waits = list(si.on_wait)
                    extra, keep = waits[:-limit], waits[-limit:]
                    for i in range(0, len(extra), limit):
                        chunk_w = extra[i:i + limit]
                        nop = mybir.InstNoOp(
                            name=f"WSPLIT-{counter[0]}", ins=[], outs=[]
                        )
                        counter[0] += 1
                        nop.engine = inst.engine
                        nop.sync_info = mybir.SyncInfo(
                            on_wait=chunk_w, on_update=[]
                        )
                        new.append(nop)
                    si.on_wait = keep
                    changed = True
                new.append(inst)
            if changed:
                blk.instructions = new
    return counter[0]


def _build_bass(dm=DM, isl=ISL, tokens=T, n_batch=B, group=G, chunk=CHUNK,
                repeat=1, psa_bufs=4, psc_bufs=3, z_first=False,
                interleave=True):
    """Build the per-core Bass module.

    Inputs (per core):
      hid_t  (dm, tokens)  bf16 : hidden_states, transposed
      w_in_t (dm, 2*isl)   bf16 : in_proj weight shard, transposed;
                                  cols [0,isl) are x-rows (pre-scaled by
                                  0.1), cols [isl,2*isl) are z-rows
      w_out_t(isl, dm)     bf16 : out_proj weight shard, transposed
      a_vec  (128, isl/128) f32 : per-inner-dim decay A = exp(-|A_log|)
    Output:
      out_t  (dm, tokens)  f32 : partial out-projection, transposed
    """
    import concourse.bass as bass
    import concourse.tile as tile
    import concourse.mybir as mybir

    _patch_tile_drain()

    ka = dm // 128            # in_proj contraction tiles
    kc = isl // 128           # out_proj contraction tiles / x slabs
    ng = tokens // group      # token groups
    gpb = ng // n_batch       # groups per batch
    kh = ka // 2              # k-tiles per hidden half-slab
    cpg = group // chunk      # chunks per group

    nc = bass.Bass("TRN2")
    dt = mybir.dt

    hid = nc.dram_tensor("hid_t", (dm, tokens), dt.bfloat16, kind="ExternalInput")
    w_in = nc.dram_tensor("w_in_t", (dm, 2 * isl), dt.bfloat16, kind="ExternalInput")
    w_out = nc.dram_tensor("w_out_t", (isl, dm), dt.bfloat16, kind="ExternalInput")
    a_vec = nc.dram_tensor("a_vec", (128, kc), dt.float32, kind="ExternalInput")
    out = nc.dram_tensor("out_t", (dm, tokens), dt.float32, kind="ExternalOutput")

    hid_r = hid[:].rearrange("(k p) t -> p k t", p=128)
    win_r = w_in[:].rearrange("(k p) m -> p k m", p=128)
    wout_r = w_out[:].rearrange("(k p) n -> p k n", p=128)

    with tile.TileContext(nc) as tc:
        with (
            tc.tile_pool(name="w_in", bufs=1) as p_win,
            tc.tile_pool(name="w_out", bufs=1) as p_wout,
            tc.tile_pool(name="consts", bufs=1) as p_const,
            tc.tile_pool(name="hid", bufs=2) as p_hid,
            tc.tile_pool(name="ysb", bufs=2) as p_y,
            tc.tile_pool(name="hstate", bufs=1) as p_h,
            tc.tile_pool(name="sig", bufs=2) as p_sig,
            tc.tile_pool(name="oev", bufs=2) as p_oev,
            tc.tile_pool(name="psA", bufs=psa_bufs, space="PSUM") as p_psA,
            tc.tile_pool(name="psC", bufs=psc_bufs, space="PSUM") as p_psC,
        ):
            win_sb = p_win.tile([128, ka, 2 * isl], dt.bfloat16)
            for k in range(ka):
                nc.sync.dma_start(out=win_sb[:, k, :], in_=win_r[:, k, :])
            wout_sb = p_wout.tile([128, kc, dm], dt.bfloat16)
            for k in range(kc):
                nc.sync.dma_start(out=wout_sb[:, k, :], in_=wout_r[:, k, :])
            a_sb = p_const.tile([128, kc], dt.float32)
            nc.sync.dma_start(out=a_sb[:], in_=a_vec[:])
            h_sb = p_h.tile([128, kc, chunk], dt.float32)

            import contextlib
            rep_ctx = (
                tc.For_i(0, repeat, 1) if repeat > 1 else contextlib.nullcontext()
            )
            with rep_ctx:
                _emit_groups(
                    nc, tc, mybir, ng, gpb, ka, kh, kc, cpg, dm, isl, group, chunk,
                    hid_r, out, p_hid, p_y, p_sig, p_oev, p_psA, p_psC,
                    win_sb, wout_sb, a_sb, h_sb, z_first=z_first,
                    interleave=interleave,
                )

    n_split = _split_excess_waits(nc)
    if n_split:
        print(f"_split_excess_waits: inserted {n_split} NOPs", flush=True)
    return nc


def _emit_groups(nc, tc, mybir, ng, gpb, ka, kh, kc, cpg, dm, isl, group, chunk,
                 hid_r, out, p_hid, p_y, p_sig, p_oev, p_psA, p_psC,
                 win_sb, wout_sb, a_sb, h_sb, pipeline=False, z_first=False,
                 interleave=False):
    # pipeline=True (out_proj of group g-1 emitted after in_proj of group
    # g) measured model-neutral and hung on hardware — keep it off.
    # interleave=True spreads group g-1's out_proj tiles between the slab
    # chains of group g so the PE never waits on the DVE scan/gate tail.
    dt = mybir.dt
    mt = dm // 128            # out_proj tiles per group = 28
    mps = mt // kc            # out_proj tiles interleaved per slab = 4

    def emit_phase_c(g, y_sb, m_lo, m_hi):
        for m in range(m_lo, m_hi):
            po = p_psC.tile([128, group], dt.float32, tag="po")
            for k in range(kc):
                nc.tensor.matmul(
                    po,
                    wout_sb[:, k, m * 128:(m + 1) * 128],
                    y_sb[:, k, :],
                    start=(k == 0),
                    stop=(k == kc - 1),
                )
            oev = p_oev.tile([128, group], dt.float32, tag="oev")
            nc.vector.tensor_copy(oev[:], po[:])
            nc.sync.dma_start(
                out=out[m * 128:(m + 1) * 128, g * group:(g + 1) * group],
                in_=oev[:],
            )

    pending_c = None
    if True:
        if True:
            for g in range(ng):
                if g % gpb == 0:
                    # scan state resets at each batch boundary
                    nc.vector.memset(h_sb[:], 0.0)

                hid_t = []
                for hh in range(2):
                    ht = p_hid.tile([128, kh, group], dt.bfloat16, tag="hid")
                    nc.sync.dma_start(
                        out=ht[:],
                        in_=hid_r[:, hh * kh:(hh + 1) * kh, g * group:(g + 1) * group],
                    )
                    hid_t.append(ht)

                y_sb = p_y.tile([128, kc, group], dt.bfloat16)

                for s in range(kc):
                    px = p_psA.tile([128, group], dt.float32, tag="ps")
                    pz = p_psA.tile([128, group], dt.float32, tag="ps")

                    def emit_x():
                        for k in range(ka):
                            hh, kk = divmod(k, kh)
                            nc.tensor.matmul(
                                px,
                                win_sb[:, k, s * 128:(s + 1) * 128],
                                hid_t[hh][:, kk, :],
                                start=(k == 0),
                                stop=(k == ka - 1),
                            )

                    def emit_z():
                        for k in range(ka):
                            hh, kk = divmod(k, kh)
                            nc.tensor.matmul(
                                pz,
                                win_sb[:, k, isl + s * 128:isl + (s + 1) * 128],
                                hid_t[hh][:, kk, :],
                                start=(k == 0),
                                stop=(k == ka - 1),
                            )

                    if z_first:
                        emit_z()
                        emit_x()
                    else:
                        emit_x()
                        emit_z()
                    sig = p_sig.tile([128, group], dt.bfloat16, tag="sig")
                    nc.scalar.activation(
                        sig[:], pz[:], mybir.ActivationFunctionType.Sigmoid
                    )
                    for c in range(cpg):
                        cs = slice(c * chunk, (c + 1) * chunk)
                        nc.vector.scalar_tensor_tensor(
                            out=h_sb[:, s, :],
                            in0=h_sb[:, s, :],
                            scalar=a_sb[:, s:s + 1],
                            in1=px[:, cs],
                            op0=mybir.AluOpType.mult,
                            op1=mybir.AluOpType.add,
                        )
                        nc.vector.tensor_mul(y_sb[:, s, cs], h_sb[:, s, :], sig[:, cs])

                    if interleave and pending_c is not None:
                        emit_phase_c(*pending_c, s * mps, (s + 1) * mps)

                if interleave:
                    pending_c = (g, y_sb)
                elif pipeline:
                    # depth-1 software pipeline: group g-1's out_proj runs
                    # after group g's in_proj on the PE, so the PE never
                    # waits on the DVE scan/gate chain.
                    if pending_c is not None:
                        emit_phase_c(*pending_c, 0, mt)
                    pending_c = (g, y_sb)
                else:
                    emit_phase_c(g, y_sb, 0, mt)
            if pending_c is not None:
                emit_phase_c(*pending_c, 0, mt)
                pending_c = None


# Results of the most recent device run (for test harness introspection).
last_result = None


def _prep_core_inputs(hidden_states, W_in, W_out, A_log):
    """Host-side shard prep. Returns (in_maps, hid_t shared array)."""
    hid_t = np.ascontiguousarray(hidden_states.reshape(T, DM).T).astype(BF16)
    A_full = np.exp(-np.abs(A_log)).astype(np.float32)  # (H,)

    in_maps = []
    for c in range(N_CORES):
        isl_sl = slice(c * ISL, (c + 1) * ISL)
        # fold the 0.1 scan input scale into the x-rows of W_in
        w_x = W_in[:INNER][isl_sl] * np.float32(0.1)
        w_z = W_in[INNER:][isl_sl]
        w_in_t = np.ascontiguousarray(
            np.concatenate([w_x, w_z], axis=0).T
        ).astype(BF16)  # (DM, 2*ISL)
        w_out_t = np.ascontiguousarray(W_out[:, isl_sl].T).astype(BF16)  # (ISL, DM)
        a_col = np.repeat(A_full[c * HPC:(c + 1) * HPC], DH)  # (ISL,)
        a_vec = np.ascontiguousarray(
            a_col.reshape(ISL // 128, 128).T
        ).astype(np.float32)  # (128, ISL/128)
        in_maps.append(
            {
                "hid_t": hid_t,
                "w_in_t": w_in_t,
                "w_out_t": w_out_t,
                "a_vec": a_vec,
            }
        )
    return in_maps


def kernel(hidden_states, W_in, W_out, A_log):
    from concourse.bass_utils import run_bass_kernel_spmd

    global last_result

    if "nc" not in _nc_cache:
        _nc_cache["nc"] = _build_bass()
    nc = _nc_cache["nc"]

    hidden_states = np.asarray(hidden_states, dtype=np.float32)
    W_in = np.asarray(W_in, dtype=np.float32)
    W_out = np.asarray(W_out, dtype=np.float32)
    A_log = np.asarray(A_log, dtype=np.float32)

    in_maps = _prep_core_inputs(hidden_states, W_in, W_out, A_log)

    last_result = run_bass_kernel_spmd(nc, in_maps, core_ids=list(range(N_CORES)))

    acc = np.zeros((DM, T), dtype=np.float32)
    for r in last_result.results:
        acc += r["out_t"]
    return np.ascontiguousarray(acc.T).reshape(B, S, DM)


if __name__ == "__main__":
    rng = np.random.default_rng(0)
    ins = {
        "hidden_states": rng.standard_normal((B, S, DM), dtype=np.float32),
        "W_in": (rng.standard_normal((2 * INNER, DM), dtype=np.float32) * 0.02),
        "W_out": (rng.standard_normal((DM, INNER), dtype=np.float32) * 0.02),
        "A_log": rng.standard_normal((H,), dtype=np.float32),
    }
    out = kernel(**ins)
    print(out.shape, out.dtype)

